# revision 5
# baseline (speedup 1.0000x reference)
"""Trainium2 Bass kernel for nn_Attention_40785009443452.

Reference computation (per batch b):
    qkv = w_qkv @ x_b            # 1x1x1 conv == channel linear
    q,k,v split into 4 heads of dim 16, tokens N = 16*16*16 = 4096
    q,k L2-normalized along head dim
    attn = softmax(q @ k^T)      # [N, N] per (b, head)
    out  = attn @ v  (+ x residual)

Sharding: 8 (batch, head) pairs -> 8 NeuronCores (data + head parallel).
Each core computes one full 4096x4096 attention.

Device algorithm (per core), S^T orientation so softmax reduction (over
keys) lands on the PSUM partition axis and is folded into the PV matmul
via an appended ones-column on V:

    B  = Wq^T Wk                     [64, 64]   (tiny matmul on device)
    G' = (B^T X) * rq  (col scale)   rq[n] = 1/||q_n||
    X' = X * rk                      rk[m] = 1/||k_m||
    S^T chunk [128 keys, 1024 qry] = X'^T(keys) @ G'(cols)  2x concurrent
                                     K=64 matmuls on PE row groups 0/64
    P^T = exp(S^T)                   ACT, batched over chunk PAIRS (FD=2048)
    O'a [33, 512]  += V'_j^T P^T     queries 0-511,   PE col group 0
    O'b [33, 512]  += V'_j^T P^T     queries 512-1023, PE col group 64
                                     (concurrent col-split PV matmuls)
    out^T = O'[0:16] / O'[32] + x_res

Main-loop PSUM: S triple-buffer [128,3,1024] (6 banks) + two PV
accumulators (2 banks) = exactly 8 banks.  exp() consumes buffer PAIRS
where contiguous (pattern per 6 chunks: pair(0,1), single(2), single(0),
pair(1,2)) to amortize the ~350cy ACT per-instruction overhead.

Normalization scales via exp(-0.5*ln(sumsq)) on ACT (Rsqrt/Reciprocal
activations banned for accuracy; Ln+Exp live in one ACT table set).
"""

import os

import numpy as np

import concourse.bass as bass
import concourse.mybir as mybir
import concourse.tile as tile
from concourse import bacc
from concourse.bass_utils import run_bass_kernel_spmd

NCORES = 8
C = 64          # channels
HEADS = 4
HD = 16         # head dim
N = 4096        # tokens (16*16*16)
NBQ = 1024      # queries per block
NB = N // NBQ   # 4 blocks
KC = 128        # keys per chunk
JT = N // KC    # 32 key chunks
FP = mybir.dt.float32
BF = mybir.dt.bfloat16

AF = mybir.ActivationFunctionType

# env kill-switches for risky features
K_PAIR = os.environ.get("K_PAIR", "1") == "1"       # paired-buffer exp
K_PVSPLIT = os.environ.get("K_PVSPLIT", "1") == "1"  # col-group PV split


def build_program():
    nc = bacc.Bacc(
        "TRN2", target_bir_lowering=False, debug=False, enable_asserts=False
    )
    x_d = nc.dram_tensor("x", [C, N], FP, kind="ExternalInput").ap()
    w_d = nc.dram_tensor("w", [3 * HD, C], FP, kind="ExternalInput").ap()
    wT_d = nc.dram_tensor("wT", [C, 3 * HD], FP, kind="ExternalInput").ap()
    xr_d = nc.dram_tensor("xres", [HD, N], FP, kind="ExternalInput").ap()
    op_d = nc.dram_tensor("onespat", [2 * HD, 33], FP,
                          kind="ExternalInput").ap()
    out_d = nc.dram_tensor("out", [HD, N], FP, kind="ExternalOutput").ap()

    with tile.TileContext(nc) as tc:
        _body(tc, x_d, w_d, wT_d, xr_d, op_d, out_d)
    nc.compile()
    return nc


def _body(tc, x_d, w_d, wT_d, xr_d, op_d, out_d):
    nc = tc.nc
    import contextlib

    # Pre-load the ACT table set containing Exp, Ln AND Square so the
    # compiler's per-function chooser doesn't flip-flop between sets.
    if os.environ.get("K_PRELOAD", "1") == "1":
        from concourse.hw_specs import get_activation_tables

        set_names = list(get_activation_tables(nc.m.arch).keys())
        set_id = set_names.index("natural_log_exp_and_others")
        nc.scalar.add_instruction(
            mybir.InstLoadActFuncSet(
                name=f"I-{nc.next_id()}", act_func_set_id=set_id
            )
        )

    with contextlib.ExitStack() as ctx:
        consts = ctx.enter_context(tc.tile_pool(name="consts", bufs=1))

        # ---- load inputs -------------------------------------------------
        wq_eng = nc.gpsimd
        WT = consts.tile([C, 3 * HD], FP)
        wq_eng.dma_start(WT, wT_d)
        Wq = consts.tile([HD, C], FP)
        wq_eng.dma_start(Wq, w_d[0:HD, :])
        Wk = consts.tile([HD, C], FP)
        wq_eng.dma_start(Wk, w_d[HD : 2 * HD, :])
        X = consts.tile([C, N], FP)
        for c8 in range(8):
            sl = slice(c8 * 512, c8 * 512 + 512)
            nc.sync.dma_start(X[:, sl], x_d[:, sl])
        XR = consts.tile([HD, N], FP)
        wq_eng.dma_start(XR, xr_d)

        ones1_16 = consts.tile([1, HD], BF)
        nc.any.memset(ones1_16, 1.0)
        ones33 = consts.tile([33, 2 * C], BF)
        nc.any.memset(ones33, 1.0)
        eps33 = consts.tile([33, 1], FP)
        nc.any.memset(eps33, 1e-24)
        ones_pat_f = consts.tile([2 * HD, 33], FP)
        nc.sync.dma_start(ones_pat_f, op_d)
        ones_pat = consts.tile([2 * HD, 33], BF)
        nc.vector.tensor_copy(ones_pat, ones_pat_f)

        Bsb2 = consts.tile([C, 2 * C], FP)     # [B | B] fp32 stationary
        Gp2 = consts.tile([2 * C, N], BF)      # (B^T X)*rq duplicated rows
        Xp2 = consts.tile([2 * C, N], BF)      # X*rk duplicated rows
        # [V_j(16) | zeros | ones@32] stationary tiles for the PV matmul;
        # ones column lands the softmax denominator on PSUM partition 32/96.
        Vp = consts.tile([KC, JT, 33], BF)
        nc.any.memset(Vp, 0.0)
        nc.any.memset(Vp[:, :, 32], 1.0)

        # ---- prologue: B, V' tiles, norms, G', X' ------------------------
        with contextlib.ExitStack() as mctx:
            pps = mctx.enter_context(
                tc.tile_pool(name="prol_ps", bufs=3, space="PSUM"))
            psb = mctx.enter_context(tc.tile_pool(name="prol_sb", bufs=2))

            # B = Wq^T Wk (fp32), duplicated into [B | B]
            ps_b = pps.tile([C, C], FP, tag="pp")
            nc.tensor.matmul(ps_b, Wq, Wk, start=True, stop=True)
            nc.vector.tensor_copy(Bsb2[:, 0:C], ps_b)
            nc.vector.tensor_copy(Bsb2[:, C : 2 * C], ps_b)

            for c4 in range(4):
                sl = slice(c4 * 1024, c4 * 1024 + 1024)

                # sumsq of q (-> partition 0) and k (-> partition 32)
                ps_q = pps.tile([2 * HD, 1024], FP, tag="pp")
                for h2 in range(2):
                    hsl = slice(h2 * 512, h2 * 512 + 512)
                    xsl = slice(c4 * 1024 + h2 * 512,
                                c4 * 1024 + h2 * 512 + 512)
                    nc.tensor.matmul(ps_q[:, hsl], WT[:, 0 : 2 * HD],
                                     X[:, xsl], start=True, stop=True)
                sqq = psb.tile([2 * HD, 1024], BF, tag="sq")
                nc.scalar.activation(sqq, ps_q, AF.Square)
                ps_nq = pps.tile([33, 1024], FP, tag="pp")
                for h2 in range(2):
                    hsl = slice(h2 * 512, h2 * 512 + 512)
                    nc.tensor.matmul(ps_nq[:, hsl], ones_pat, sqq[:, hsl],
                                     start=True, stop=True)
                # rq on partition 0, rk on partition 32 (extra rows wasted)
                lnq = psb.tile([33, 1024], FP, tag="ln")
                nc.scalar.activation(lnq, ps_nq, AF.Ln, bias=eps33)
                rqk = psb.tile([33, 1024], BF, tag="rqk")
                nc.scalar.activation(rqk, lnq, AF.Exp, scale=-0.5)

                # replicate rq/rk across 128 partitions via K=1 matmuls
                ps_rep = pps.tile([2 * C, 1024], FP, tag="pp")
                ps_repk = pps.tile([2 * C, 1024], FP, tag="pp")
                ps_g = pps.tile([2 * C, 1024], FP, tag="pp")
                for h2 in range(2):
                    hsl = slice(h2 * 512, h2 * 512 + 512)
                    xsl = slice(c4 * 1024 + h2 * 512,
                                c4 * 1024 + h2 * 512 + 512)
                    nc.tensor.matmul(ps_rep[:, hsl], ones33[0:1, :],
                                     rqk[0:1, hsl], start=True, stop=True)
                    nc.tensor.matmul(ps_repk[:, hsl], ones33[32:33, :],
                                     rqk[32:33, hsl], start=True, stop=True)
                    nc.tensor.matmul(ps_g[:, hsl], Bsb2, X[:, xsl],
                                     start=True, stop=True)
                rep_sb = psb.tile([2 * C, 1024], BF, tag="rep")
                nc.vector.tensor_copy(rep_sb, ps_rep)
                nc.vector.tensor_mul(Gp2[:, sl], ps_g, rep_sb)
                nc.vector.tensor_mul(Xp2[0:C, sl], ps_repk[0:C, :],
                                     X[:, sl])
                nc.vector.tensor_mul(Xp2[C : 2 * C, sl],
                                     ps_repk[C : 2 * C, :], X[:, sl])

                # V' tiles for this chunk's 8 key ranges
                for j in range(8 * c4, 8 * c4 + 8):
                    ksl = slice(j * KC, j * KC + KC)
                    ps_kv = pps.tile([KC, HD], FP, tag="ppv", bufs=2)
                    nc.tensor.matmul(ps_kv, X[:, ksl],
                                     WT[:, 2 * HD : 3 * HD],
                                     start=True, stop=True)
                    nc.vector.tensor_copy(Vp[:, j, 0:HD], ps_kv)

        # ---- main attention loop ----------------------------------------
        with contextlib.ExitStack() as mctx:
            ps_pool = mctx.enter_context(
                tc.tile_pool(name="ps_main", bufs=1, space="PSUM"))
            pt_pool = mctx.enter_context(tc.tile_pool(name="pt", bufs=2))
            ep_pool = mctx.enter_context(tc.tile_pool(name="ep", bufs=2))

            # S^T triple buffer: [128 keys, slot, 1024 queries] = 6 banks
            ps_big = ps_pool.tile([KC, 3, NBQ], FP, tag="ps_big")

            def epilogue(nb, po_a, po_b):
                nbase = nb * NBQ
                oall = ep_pool.tile([33, NBQ], FP, tag="oall",
                                    name=f"oall_{nb}")
                nc.vector.tensor_copy(oall[:, 0:512], po_a)
                nc.vector.tensor_copy(oall[:, 512:1024], po_b[C:97, :])
                lnd = ep_pool.tile([1, NBQ], FP, tag="lnd", name=f"lnd_{nb}")
                nc.scalar.activation(lnd, oall[32:33, :], AF.Ln)
                rinv = ep_pool.tile([1, NBQ], BF, tag="rinv",
                                    name=f"rinv_{nb}")
                nc.scalar.activation(rinv, lnd, AF.Exp, scale=-1.0)
                t2 = ep_pool.tile([HD, NBQ], FP, tag="t2", name=f"t2_{nb}")
                for h2 in range(2):
                    qsl = slice(h2 * 512, h2 * 512 + 512)
                    tag = "po_a" if h2 == 0 else "po_b"
                    shape = [HD, 512] if h2 == 0 else [C + HD, 512]
                    ps_rep = ps_pool.tile(shape, FP, tag=tag,
                                          name=f"ps_rep_{nb}_{h2}")
                    pr = ps_rep if h2 == 0 else ps_rep[C : C + HD, :]
                    nc.tensor.matmul(pr, ones1_16, rinv[:, qsl],
                                     start=True, stop=True)
                    nc.vector.tensor_mul(t2[:, qsl], oall[0:HD, qsl], pr)
                osb = ep_pool.tile([HD, NBQ], FP, tag="osb",
                                   name=f"osb_{nb}")
                osl = slice(nbase, nbase + NBQ)
                nc.vector.tensor_add(osb, t2, XR[:, osl])
                nc.sync.dma_start(out_d[:, osl], osb)

            def s_matmul(nb, j):
                nbase = nb * NBQ
                t = j % 3
                ksl = slice(j * KC, j * KC + KC)
                for h2 in range(2):
                    qsl = slice(h2 * 512, h2 * 512 + 512)
                    gsl = slice(nbase + h2 * 512, nbase + h2 * 512 + 512)
                    rg = slice(h2 * C, h2 * C + C)  # alternate row groups
                    nc.tensor.matmul(ps_big[:, t, qsl], Xp2[rg, ksl],
                                     Gp2[rg, gsl], start=True, stop=True)

            def pv_matmul(j, pt_ap, po_a, po_b):
                first, last = j == 0, j == JT - 1
                nc.tensor.matmul(po_a, Vp[:, j, :], pt_ap[:, 0:512],
                                 start=first, stop=last)
                if K_PVSPLIT:
                    nc.tensor.matmul(po_b[C:97, :], Vp[:, j, :],
                                     pt_ap[:, 512:1024],
                                     start=first, stop=last)
                else:
                    nc.tensor.matmul(po_b[0:33, :], Vp[:, j, :],
                                     pt_ap[:, 512:1024],
                                     start=first, stop=last)

            for nb in range(NB):
                po_a = ps_pool.tile([33, 512], FP, tag="po_a",
                                    name=f"po_a_{nb}")
                po_b = ps_pool.tile([97, 512], FP, tag="po_b",
                                    name=f"po_b_{nb}")
                j = 0
                while j < JT:
                    t = j % 3
                    # slots (0,1) are consumed by paired exps (FD=2048),
                    # slot 2 by singles: pattern P(0,1) S(2) P(0,1) S(2) ...
                    pair = K_PAIR and t < 2 and j + 1 < JT
                    if pair:
                        s_matmul(nb, j)
                        s_matmul(nb, j + 1)
                        pt = pt_pool.tile([KC, 2, NBQ], BF, tag="ptp")
                        nc.scalar.activation(pt, ps_big[:, t : t + 2, :],
                                             AF.Exp)
                        pv_matmul(j, pt[:, 0, :], po_a, po_b)
                        pv_matmul(j + 1, pt[:, 1, :], po_a, po_b)
                        j += 2
                    else:
                        s_matmul(nb, j)
                        pt = pt_pool.tile([KC, NBQ], BF, tag="pts")
                        nc.scalar.activation(pt, ps_big[:, t, :], AF.Exp)
                        pv_matmul(j, pt, po_a, po_b)
                        j += 1
                # epilogue immediately: the PV accumulators are single-
                # buffered, so their readers must be emitted before the
                # next block's PV writes (emission order = dep order).
                epilogue(nb, po_a, po_b)


_CACHE = {}


def _get_program():
    if "nc" not in _CACHE:
        _CACHE["nc"] = build_program()
    return _CACHE["nc"]


def make_in_maps(x, w_qkv):
    """Shard full inputs into per-core input maps. Core i = (b=i//4, h=i%4)."""
    x = np.ascontiguousarray(np.asarray(x, dtype=np.float32))
    w_qkv = np.ascontiguousarray(np.asarray(w_qkv, dtype=np.float32))
    b_, c, d, hh, ww = x.shape
    xf = x.reshape(b_, c, d * hh * ww)
    in_maps = []
    for core in range(NCORES):
        b, h = divmod(core, HEADS)
        rows = np.concatenate([
            np.arange(h * HD, (h + 1) * HD),
            np.arange(C + h * HD, C + (h + 1) * HD),
            np.arange(2 * C + h * HD, 2 * C + (h + 1) * HD),
        ])
        w_h = np.ascontiguousarray(w_qkv[rows, :])          # [48, 64]
        wT_h = np.ascontiguousarray(w_h.T)                   # [64, 48]
        x_b = np.ascontiguousarray(xf[b])                    # [64, 4096]
        x_res = np.ascontiguousarray(x_b[h * HD : (h + 1) * HD])  # [16, 4096]
        # col 0 sums q squares -> partition 0; col 32 sums k squares ->
        # partition 32 (PSUM reads must start 32-aligned)
        ones_pat = np.zeros((2 * HD, 33), dtype=np.float32)
        ones_pat[0:HD, 0] = 1.0
        ones_pat[HD : 2 * HD, 32] = 1.0
        in_maps.append({"x": x_b, "w": w_h, "wT": wT_h, "xres": x_res,
                        "onespat": ones_pat})
    return in_maps


def assemble_output(results, x_shape):
    b_, c, d, hh, ww = x_shape
    out = np.empty((b_, c, d * hh * ww), dtype=np.float32)
    for core in range(NCORES):
        b, h = divmod(core, HEADS)
        out[b, h * HD : (h + 1) * HD] = results[core]["out"]
    return out.reshape(x_shape)


def run(x, w_qkv, trace=False, **kw):
    nc = _get_program()
    in_maps = make_in_maps(x, w_qkv)
    res = run_bass_kernel_spmd(nc, in_maps, list(range(NCORES)),
                               trace=trace, **kw)
    return assemble_output(res.results, np.asarray(x).shape), res


def kernel(x, w_qkv):
    out, _ = run(x, w_qkv)
    return out


# revision 6
# speedup vs baseline: 1.1977x; 1.1977x over previous
"""Trainium2 Bass kernel for nn_Attention_40785009443452.

Reference computation (per batch b):
    qkv = w_qkv @ x_b            # 1x1x1 conv == channel linear
    q,k,v split into 4 heads of dim 16, tokens N = 16*16*16 = 4096
    q,k L2-normalized along head dim
    attn = softmax(q @ k^T)      # [N, N] per (b, head)
    out  = attn @ v  (+ x residual)

Sharding: 8 (batch, head) pairs -> 8 NeuronCores (data + head parallel).
Each core computes one full 4096x4096 attention.

Device algorithm (per core), S^T orientation so softmax reduction (over
keys) lands on the PSUM partition axis and is folded into the PV matmul
via an appended ones-column on V:

    B  = Wq^T Wk                     [64, 64]   (tiny matmul on device)
    G' = (B^T X) * rq  (col scale)   rq[n] = 1/||q_n||
    X' = X * rk                      rk[m] = 1/||k_m||
    S^T chunk [128 keys, 1024 qry] = X'^T(keys) @ G'(cols)  2x concurrent
                                     K=64 matmuls on PE row groups 0/64
    P^T = exp(S^T)                   ACT, batched over chunk PAIRS (FD=2048)
    O'a [33, 512]  += V'_j^T P^T     queries 0-511,   PE col group 0
    O'b [33, 512]  += V'_j^T P^T     queries 512-1023, PE col group 64
                                     (concurrent col-split PV matmuls)
    out^T = O'[0:16] / O'[32] + x_res

Main-loop PSUM: S triple-buffer [128,3,1024] (6 banks) + two PV
accumulators (2 banks) = exactly 8 banks.  exp() consumes buffer PAIRS
where contiguous (pattern per 6 chunks: pair(0,1), single(2), single(0),
pair(1,2)) to amortize the ~350cy ACT per-instruction overhead.

Normalization scales via exp(-0.5*ln(sumsq)) on ACT (Rsqrt/Reciprocal
activations banned for accuracy; Ln+Exp live in one ACT table set).
"""

import os

import numpy as np

import concourse.bass as bass
import concourse.mybir as mybir
import concourse.tile as tile
from concourse import bacc
from concourse.bass_utils import run_bass_kernel_spmd

NCORES = 8
C = 64          # channels
HEADS = 4
HD = 16         # head dim
N = 4096        # tokens (16*16*16)
NBQ = 1024      # queries per block
NB = N // NBQ   # 4 blocks
KC = 128        # keys per chunk
JT = N // KC    # 32 key chunks
FP = mybir.dt.float32
BF = mybir.dt.bfloat16

AF = mybir.ActivationFunctionType

# env kill-switches for risky features
K_PAIR = os.environ.get("K_PAIR", "1") == "1"       # paired-buffer exp
K_PVSPLIT = os.environ.get("K_PVSPLIT", "1") == "1"  # col-group PV split


def build_program():
    nc = bacc.Bacc(
        "TRN2", target_bir_lowering=False, debug=False, enable_asserts=False
    )
    x_d = nc.dram_tensor("x", [C, N], FP, kind="ExternalInput").ap()
    w_d = nc.dram_tensor("w", [3 * HD, C], FP, kind="ExternalInput").ap()
    wT_d = nc.dram_tensor("wT", [C, 3 * HD], FP, kind="ExternalInput").ap()
    xr_d = nc.dram_tensor("xres", [HD, N], FP, kind="ExternalInput").ap()
    op_d = nc.dram_tensor("onespat", [2 * HD, 33], FP,
                          kind="ExternalInput").ap()
    out_d = nc.dram_tensor("out", [HD, N], FP, kind="ExternalOutput").ap()

    with tile.TileContext(nc) as tc:
        _body(tc, x_d, w_d, wT_d, xr_d, op_d, out_d)
    nc.compile()
    return nc


def _body(tc, x_d, w_d, wT_d, xr_d, op_d, out_d):
    nc = tc.nc
    import contextlib

    # Pre-load the ACT table set containing Exp, Ln AND Square so the
    # compiler's per-function chooser doesn't flip-flop between sets.
    if os.environ.get("K_PRELOAD", "1") == "1":
        from concourse.hw_specs import get_activation_tables

        set_names = list(get_activation_tables(nc.m.arch).keys())
        set_id = set_names.index("natural_log_exp_and_others")
        nc.scalar.add_instruction(
            mybir.InstLoadActFuncSet(
                name=f"I-{nc.next_id()}", act_func_set_id=set_id
            )
        )

    with contextlib.ExitStack() as ctx:
        consts = ctx.enter_context(tc.tile_pool(name="consts", bufs=1))

        # ---- load inputs -------------------------------------------------
        wq_eng = nc.gpsimd
        WT = consts.tile([C, 3 * HD], FP)
        wq_eng.dma_start(WT, wT_d)
        Wq = consts.tile([HD, C], FP)
        wq_eng.dma_start(Wq, w_d[0:HD, :])
        Wk = consts.tile([HD, C], FP)
        wq_eng.dma_start(Wk, w_d[HD : 2 * HD, :])
        X = consts.tile([C, N], FP)
        for c8 in range(8):
            sl = slice(c8 * 512, c8 * 512 + 512)
            nc.sync.dma_start(X[:, sl], x_d[:, sl])
        XR = consts.tile([HD, N], FP)
        wq_eng.dma_start(XR, xr_d)

        ones1_16 = consts.tile([1, HD], BF)
        nc.any.memset(ones1_16, 1.0)
        ones33 = consts.tile([33, 2 * C], BF)
        nc.any.memset(ones33, 1.0)
        eps33 = consts.tile([33, 1], FP)
        nc.any.memset(eps33, 1e-24)
        ones_pat_f = consts.tile([2 * HD, 33], FP)
        nc.sync.dma_start(ones_pat_f, op_d)
        ones_pat = consts.tile([2 * HD, 33], BF)
        nc.vector.tensor_copy(ones_pat, ones_pat_f)

        Bsb2 = consts.tile([C, 2 * C], FP)     # [B | B] fp32 stationary
        Gp2 = consts.tile([2 * C, N], BF)      # (B^T X)*rq duplicated rows
        Xp2 = consts.tile([2 * C, N], BF)      # X*rk duplicated rows
        # [V_j(16) | zeros | ones@32] stationary tiles for the PV matmul;
        # ones column lands the softmax denominator on PSUM partition 32/96.
        Vp = consts.tile([KC, JT, 33], BF)
        nc.any.memset(Vp, 0.0)
        nc.any.memset(Vp[:, :, 32], 1.0)

        # ---- prologue: B, V' tiles, norms, G', X' ------------------------
        with contextlib.ExitStack() as mctx:
            pps = mctx.enter_context(
                tc.tile_pool(name="prol_ps", bufs=3, space="PSUM"))
            psb = mctx.enter_context(tc.tile_pool(name="prol_sb", bufs=2))

            # B = Wq^T Wk (fp32), duplicated into [B | B]
            ps_b = pps.tile([C, C], FP, tag="pp")
            nc.tensor.matmul(ps_b, Wq, Wk, start=True, stop=True)
            nc.vector.tensor_copy(Bsb2[:, 0:C], ps_b)
            nc.vector.tensor_copy(Bsb2[:, C : 2 * C], ps_b)

            for c4 in range(4):
                sl = slice(c4 * 1024, c4 * 1024 + 1024)

                # sumsq of q (-> partition 0) and k (-> partition 32)
                ps_q = pps.tile([2 * HD, 1024], FP, tag="pp")
                for h2 in range(2):
                    hsl = slice(h2 * 512, h2 * 512 + 512)
                    xsl = slice(c4 * 1024 + h2 * 512,
                                c4 * 1024 + h2 * 512 + 512)
                    nc.tensor.matmul(ps_q[:, hsl], WT[:, 0 : 2 * HD],
                                     X[:, xsl], start=True, stop=True)
                sqq = psb.tile([2 * HD, 1024], BF, tag="sq")
                nc.scalar.activation(sqq, ps_q, AF.Square)
                ps_nq = pps.tile([33, 1024], FP, tag="pp")
                for h2 in range(2):
                    hsl = slice(h2 * 512, h2 * 512 + 512)
                    nc.tensor.matmul(ps_nq[:, hsl], ones_pat, sqq[:, hsl],
                                     start=True, stop=True)
                # rq on partition 0, rk on partition 32 (extra rows wasted)
                lnq = psb.tile([33, 1024], FP, tag="ln")
                nc.scalar.activation(lnq, ps_nq, AF.Ln, bias=eps33)
                rqk = psb.tile([33, 1024], BF, tag="rqk")
                nc.scalar.activation(rqk, lnq, AF.Exp, scale=-0.5)

                # replicate rq/rk across 128 partitions via K=1 matmuls
                ps_rep = pps.tile([2 * C, 1024], FP, tag="pp")
                ps_repk = pps.tile([2 * C, 1024], FP, tag="pp")
                ps_g = pps.tile([2 * C, 1024], FP, tag="pp")
                for h2 in range(2):
                    hsl = slice(h2 * 512, h2 * 512 + 512)
                    xsl = slice(c4 * 1024 + h2 * 512,
                                c4 * 1024 + h2 * 512 + 512)
                    nc.tensor.matmul(ps_rep[:, hsl], ones33[0:1, :],
                                     rqk[0:1, hsl], start=True, stop=True)
                    nc.tensor.matmul(ps_repk[:, hsl], ones33[32:33, :],
                                     rqk[32:33, hsl], start=True, stop=True)
                    nc.tensor.matmul(ps_g[:, hsl], Bsb2, X[:, xsl],
                                     start=True, stop=True)
                rep_sb = psb.tile([2 * C, 1024], BF, tag="rep")
                nc.vector.tensor_copy(rep_sb, ps_rep)
                nc.vector.tensor_mul(Gp2[:, sl], ps_g, rep_sb)
                nc.vector.tensor_mul(Xp2[0:C, sl], ps_repk[0:C, :],
                                     X[:, sl])
                nc.vector.tensor_mul(Xp2[C : 2 * C, sl],
                                     ps_repk[C : 2 * C, :], X[:, sl])

                # V' tiles for this chunk's 8 key ranges
                for j in range(8 * c4, 8 * c4 + 8):
                    ksl = slice(j * KC, j * KC + KC)
                    ps_kv = pps.tile([KC, HD], FP, tag="ppv", bufs=2)
                    nc.tensor.matmul(ps_kv, X[:, ksl],
                                     WT[:, 2 * HD : 3 * HD],
                                     start=True, stop=True)
                    nc.vector.tensor_copy(Vp[:, j, 0:HD], ps_kv)

        # ---- main attention loop ----------------------------------------
        with contextlib.ExitStack() as mctx:
            ps_pool = mctx.enter_context(
                tc.tile_pool(name="ps_main", bufs=1, space="PSUM"))
            pt_pool = mctx.enter_context(tc.tile_pool(name="pt", bufs=2))
            ep_pool = mctx.enter_context(tc.tile_pool(name="ep", bufs=2))

            # S^T triple buffer: [128 keys, slot, 1024 queries] = 6 banks
            ps_big = ps_pool.tile([KC, 3, NBQ], FP, tag="ps_big")

            def epilogue(nb, po_a, po_b):
                nbase = nb * NBQ
                oall = ep_pool.tile([33, NBQ], FP, tag="oall",
                                    name=f"oall_{nb}")
                nc.vector.tensor_copy(oall[:, 0:512], po_a)
                nc.vector.tensor_copy(oall[:, 512:1024], po_b[C:97, :])
                lnd = ep_pool.tile([1, NBQ], FP, tag="lnd", name=f"lnd_{nb}")
                nc.scalar.activation(lnd, oall[32:33, :], AF.Ln)
                rinv = ep_pool.tile([1, NBQ], BF, tag="rinv",
                                    name=f"rinv_{nb}")
                nc.scalar.activation(rinv, lnd, AF.Exp, scale=-1.0)
                t2 = ep_pool.tile([HD, NBQ], FP, tag="t2", name=f"t2_{nb}")
                for h2 in range(2):
                    qsl = slice(h2 * 512, h2 * 512 + 512)
                    tag = "po_a" if h2 == 0 else "po_b"
                    shape = [HD, 512] if h2 == 0 else [C + HD, 512]
                    ps_rep = ps_pool.tile(shape, FP, tag=tag,
                                          name=f"ps_rep_{nb}_{h2}")
                    pr = ps_rep if h2 == 0 else ps_rep[C : C + HD, :]
                    nc.tensor.matmul(pr, ones1_16, rinv[:, qsl],
                                     start=True, stop=True)
                    nc.vector.tensor_mul(t2[:, qsl], oall[0:HD, qsl], pr)
                osb = ep_pool.tile([HD, NBQ], FP, tag="osb",
                                   name=f"osb_{nb}")
                osl = slice(nbase, nbase + NBQ)
                nc.vector.tensor_add(osb, t2, XR[:, osl])
                nc.sync.dma_start(out_d[:, osl], osb)

            def s_matmul(nb, j):
                nbase = nb * NBQ
                t = j % 3
                ksl = slice(j * KC, j * KC + KC)
                for h2 in range(2):
                    qsl = slice(h2 * 512, h2 * 512 + 512)
                    gsl = slice(nbase + h2 * 512, nbase + h2 * 512 + 512)
                    rg = slice(h2 * C, h2 * C + C)  # alternate row groups
                    nc.tensor.matmul(ps_big[:, t, qsl], Xp2[rg, ksl],
                                     Gp2[rg, gsl], start=True, stop=True)

            def pv_matmul(j, pt_ap, po_a, po_b):
                first, last = j == 0, j == JT - 1
                nc.tensor.matmul(po_a, Vp[:, j, :], pt_ap[:, 0:512],
                                 start=first, stop=last)
                if K_PVSPLIT:
                    nc.tensor.matmul(po_b[C:97, :], Vp[:, j, :],
                                     pt_ap[:, 512:1024],
                                     start=first, stop=last)
                else:
                    nc.tensor.matmul(po_b[0:33, :], Vp[:, j, :],
                                     pt_ap[:, 512:1024],
                                     start=first, stop=last)

            # Group list: slots (0,1) are consumed by paired exps
            # (FD=2048), slot 2 by singles: P(0,1) S(2) P(0,1) S(2) ...
            groups = []
            for nb in range(NB):
                j = 0
                while j < JT:
                    t = j % 3
                    if K_PAIR and t < 2 and j + 1 < JT:
                        groups.append((nb, (j, j + 1)))
                        j += 2
                    else:
                        groups.append((nb, (j,)))
                        j += 1

            # Software-pipelined emission with 2-group S lookahead: the PE
            # queue is strict in-order, so exp-dependent PV matmuls must
            # not sit ahead of independent S matmuls.  Slot rotation makes
            # group g+2's S depend exactly on exp(g) having read its slots.
            po = {}

            def ensure_po(nb):
                if nb not in po:
                    po_a = ps_pool.tile([33, 512], FP, tag="po_a",
                                        name=f"po_a_{nb}")
                    po_b = ps_pool.tile([97, 512], FP, tag="po_b",
                                        name=f"po_b_{nb}")
                    po[nb] = (po_a, po_b)
                return po[nb]

            for idx, (nb, js) in enumerate(groups):
                if idx == 0:
                    for g in range(2):
                        gnb, gjs = groups[g]
                        for jj in gjs:
                            s_matmul(gnb, jj)
                # exp for this group
                t = js[0] % 3
                if len(js) == 2:
                    pt = pt_pool.tile([KC, 2, NBQ], BF, tag="ptp")
                    nc.scalar.activation(pt, ps_big[:, t : t + 2, :],
                                         AF.Exp)
                    pts = [pt[:, 0, :], pt[:, 1, :]]
                else:
                    pt = pt_pool.tile([KC, NBQ], BF, tag="pts")
                    nc.scalar.activation(pt, ps_big[:, t, :], AF.Exp)
                    pts = [pt]
                # S lookahead for group idx+2
                if idx + 2 < len(groups):
                    gnb, gjs = groups[idx + 2]
                    for jj in gjs:
                        s_matmul(gnb, jj)
                # PV for this group
                po_a, po_b = ensure_po(nb)
                for jj, pt_ap in zip(js, pts):
                    pv_matmul(jj, pt_ap, po_a, po_b)
                # epilogue immediately after a block's last PV: the PV
                # accumulators are single-buffered, so their readers must
                # be emitted before the next block's PV writes.
                if js[-1] == JT - 1:
                    epilogue(nb, po_a, po_b)


_CACHE = {}


def _get_program():
    if "nc" not in _CACHE:
        _CACHE["nc"] = build_program()
    return _CACHE["nc"]


def make_in_maps(x, w_qkv):
    """Shard full inputs into per-core input maps. Core i = (b=i//4, h=i%4)."""
    x = np.ascontiguousarray(np.asarray(x, dtype=np.float32))
    w_qkv = np.ascontiguousarray(np.asarray(w_qkv, dtype=np.float32))
    b_, c, d, hh, ww = x.shape
    xf = x.reshape(b_, c, d * hh * ww)
    in_maps = []
    for core in range(NCORES):
        b, h = divmod(core, HEADS)
        rows = np.concatenate([
            np.arange(h * HD, (h + 1) * HD),
            np.arange(C + h * HD, C + (h + 1) * HD),
            np.arange(2 * C + h * HD, 2 * C + (h + 1) * HD),
        ])
        w_h = np.ascontiguousarray(w_qkv[rows, :])          # [48, 64]
        wT_h = np.ascontiguousarray(w_h.T)                   # [64, 48]
        x_b = np.ascontiguousarray(xf[b])                    # [64, 4096]
        x_res = np.ascontiguousarray(x_b[h * HD : (h + 1) * HD])  # [16, 4096]
        # col 0 sums q squares -> partition 0; col 32 sums k squares ->
        # partition 32 (PSUM reads must start 32-aligned)
        ones_pat = np.zeros((2 * HD, 33), dtype=np.float32)
        ones_pat[0:HD, 0] = 1.0
        ones_pat[HD : 2 * HD, 32] = 1.0
        in_maps.append({"x": x_b, "w": w_h, "wT": wT_h, "xres": x_res,
                        "onespat": ones_pat})
    return in_maps


def assemble_output(results, x_shape):
    b_, c, d, hh, ww = x_shape
    out = np.empty((b_, c, d * hh * ww), dtype=np.float32)
    for core in range(NCORES):
        b, h = divmod(core, HEADS)
        out[b, h * HD : (h + 1) * HD] = results[core]["out"]
    return out.reshape(x_shape)


def run(x, w_qkv, trace=False, **kw):
    nc = _get_program()
    in_maps = make_in_maps(x, w_qkv)
    res = run_bass_kernel_spmd(nc, in_maps, list(range(NCORES)),
                               trace=trace, **kw)
    return assemble_output(res.results, np.asarray(x).shape), res


def kernel(x, w_qkv):
    out, _ = run(x, w_qkv)
    return out
